# revision 45
# baseline (speedup 1.0000x reference)
"""OHEM loss (region + affinity) on Trainium2 — 8 NeuronCores, SPMD data-parallel.

Math: for each pair (gt, pred) with shared conf_map,
    loss = (gt - pred)^2 * conf_map
    pos  = gt > 0.1 ; pos_num = sum(pos)
    neg_num = min(n - pos_num, 3 * pos_num)
    result  = (topk(neg_loss, neg_num).sum() + (loss*pos).sum()) / (neg_num + pos_num)
When neg_num == n - pos_num (the min picks the negative count, true whenever
pos fraction >= 0.25), the top-k covers every negative element, so
result == loss.sum() / n exactly. The device computes the per-shard
sum(loss) partials; the host combines them in float64, decides the min()
branch with a cheap boolean count, and falls back to an exact numpy
evaluation in the (never-taken-for-this-distribution) other branch.

Device-side design (memory-bound kernel, so bf16 halves HBM traffic):
- Host packs the five f32 shards into ONE bf16 HBM tensor per core,
  [128, sum(5*W)] with per-tile blocks [gt_r | gt_a | pred_r | pred_a | conf].
- One HWDGE DMA per tile (sync-engine trigger; all queued upfront), giving
  ~11.5KB per-partition packets — the per-packet sweet spot (>16KB packets
  drop per-engine DMA rate ~20%, and SWDGE gpsimd triggers suffer
  descriptor-ring contention on SDMA engine 15).
- Compute per tile: one fused 2W-wide DVE tensor_sub (bf16 2x mode) covers
  both pairs; pair0 squares on Act while pair1 squares on DVE (keeps the
  greedy scheduler fed with ready Vector work so it never serializes on the
  Act round-trip); DVE tensor_mul (2x) applies conf; the idle TensorE
  reduces along partitions via ones-matmul into an accumulating PSUM
  [1, CHUNK] per pair. All engines stay below the DMA stream time, so the
  kernel tracks the HBM roofline; the tapered tail tiles keep the post-DMA
  drain short.
- Output: the two PSUM accumulators, DMA'd out; host folds [2, CHUNK] x 8
  cores in f64.
"""

import os
import sys

import numpy as np

for _p in ("/opt/trn_rl_repo", os.path.expanduser("~/.axon_site/_ro/trn_rl_repo")):
    if os.path.isdir(_p) and _p not in sys.path:
        sys.path.insert(0, _p)

import ml_dtypes

import concourse.tile as tile
from concourse import bacc, mybir
from concourse.bass_utils import run_bass_kernel_spmd

B, CH, H, W = 16, 1, 768, 768
NCORES = 8
N_FULL = B * CH * H * W            # 9_437_184
N_CORE = N_FULL // NCORES          # 1_179_648
P = 128
COLS = N_CORE // P                 # 9216 free-dim columns per tensor per core
WS = (384, 1152, 1152, 1152, 1152, 1152, 1152, 1152, 768)
                                   # per-tile column widths (sum == COLS): small
                                   # tiles keep per-engine chains short; tapered
                                   # tail tiles minimize the post-DMA drain
CHUNK = 384                        # PSUM accumulator width (per-pair)
NT = len(WS)
TOT = 5 * COLS                     # packed columns per partition row (46080)
NEG_RATIO = 3.0
POS_MIN = 0.1
NAMES = ("gt_region", "pred_region", "gt_affinity", "pred_affinity", "conf_map")
# packed order: both gts adjacent, both preds adjacent -> one fused sub
PACK_ORDER = ("gt_region", "gt_affinity", "pred_region", "pred_affinity", "conf_map")
F32 = mybir.dt.float32
BF16 = mybir.dt.bfloat16
BF16NP = ml_dtypes.bfloat16

_NC_CACHE = None
LAST_RESULTS = None                # exposed for test harness profiling


def _emit(tc, big, out):
    nc = tc.nc
    wmax = max(WS)
    with (
        tc.tile_pool(name="io", bufs=9) as io_pool,
        tc.tile_pool(name="scr", bufs=4) as scr_pool,
        tc.tile_pool(name="cst", bufs=1) as cst_pool,
        tc.psum_pool(name="ps", bufs=1) as psum_pool,
    ):
        ones = cst_pool.tile([P, 1], BF16)
        nc.gpsimd.memset(ones[:], 1.0)
        ps_r = psum_pool.tile([1, CHUNK], F32, tag="ps0")
        ps_a = psum_pool.tile([1, CHUNK], F32, tag="ps1")
        ps = [ps_r, ps_a]
        started = [False, False]
        offs = [sum(5 * ww for ww in WS[:t]) for t in range(NT)]

        def load(t):
            w = WS[t]
            blk = io_pool.tile([P, 5 * wmax], BF16, tag="blk", name=f"blk{t}")
            # HWDGE (sync-engine trigger): avoids the SWDGE descriptor-ring
            # contention that slows SDMA engine 15 ~20%. Per-partition rows of
            # 5*w*2 = ~11.5KB are the per-packet DMA sweet spot.
            nc.sync.dma_start(blk[:, : 5 * w], big[:, offs[t] : offs[t] + 5 * w])
            return blk

        def sub(t, blk):
            w = WS[t]
            d = scr_pool.tile([P, 2 * wmax], BF16, tag="d", name=f"d{t}")
            nc.vector.tensor_sub(d[:, : 2 * w], blk[:, : 2 * w], blk[:, 2 * w : 4 * w])
            return d

        def squares(t, d):
            w = WS[t]
            out = []
            for pi in range(2):
                s = scr_pool.tile([P, wmax], BF16, tag=f"s{pi}", name=f"s{pi}_{t}")
                dp = d[:, pi * w : (pi + 1) * w]
                if pi == 0:
                    # pair0 squares on Act; pair1 squares on DVE. The DVE-side
                    # square+muls (~1520ns) cover Act's square latency
                    # (~1254ns), so the greedy scheduler always has ready
                    # Vector work and never serializes on the Act round-trip.
                    nc.scalar.square(s[:, :w], dp)
                else:
                    nc.vector.tensor_mul(s[:, :w], dp, dp)
                out.append(s)
            return out

        # 3-stage software pipeline (sub -> squares -> muls+matmuls), emitted
        # stage-shifted so each op's inputs were produced a full tile earlier:
        # the scheduler's per-engine queues then never stall on cross-engine
        # latency (which otherwise locks the cadence to the sub->square->mul
        # chain instead of the DMA rate).
        # all tiles buffered (io bufs == NT): queue every load upfront so the
        # HWDGE ring is never starved by trigger dependencies
        blks, ds, ss = {}, {}, {}
        for t in range(NT):
            blks[t] = load(t)
        ds[0] = sub(0, blks[0])
        ss[0] = squares(0, ds[0])
        for t, w in enumerate(WS):
            if t + 1 < NT:
                ds[t + 1] = sub(t + 1, blks[t + 1])
                ss[t + 1] = squares(t + 1, ds[t + 1])
            conf = blks[t][:, 4 * w : 5 * w]
            for pi in range(2):
                s = ss[t][pi]
                l = scr_pool.tile([P, wmax], BF16, tag=f"l{pi}", name=f"l{pi}_{t}")
                if pi == 0 and 1 <= t <= NT - 3:
                    # mid-tile pair-0 conf-muls run on the idle GpSimd engine:
                    # their only consumer is TensorE (which has slack), so no
                    # Vector op ever waits on GpSimd, and Vector's total busy
                    # drops below the DMA stream time (drains the tail
                    # backlog). Tail tiles stay on DVE to keep the final
                    # chain short.
                    nc.gpsimd.tensor_mul(l[:, :w], s[:, :w], conf)
                else:
                    nc.vector.tensor_mul(l[:, :w], s[:, :w], conf)
                nchunk = w // CHUNK
                for j in range(nchunk):
                    last = t == NT - 1 and j == nchunk - 1
                    nc.tensor.matmul(
                        ps[pi][:],
                        ones[:],
                        l[:, j * CHUNK : (j + 1) * CHUNK],
                        start=not started[pi],
                        stop=last,
                        skip_group_check=True,
                    )
                    started[pi] = True
        fin = cst_pool.tile([1, 2 * CHUNK], F32, tag="fin")
        nc.vector.tensor_copy(fin[:, 0:CHUNK], ps[0][:])
        nc.scalar.copy(fin[:, CHUNK : 2 * CHUNK], ps[1][:])
        nc.sync.dma_start(out[:], fin[:])


def _build_nc():
    nc = bacc.Bacc("TRN2", target_bir_lowering=False, debug=False, num_devices=NCORES)
    big = nc.dram_tensor("big", [P, TOT], BF16, kind="ExternalInput").ap()
    out = nc.dram_tensor("out", [1, 2 * CHUNK], F32, kind="ExternalOutput").ap()
    with tile.TileContext(nc) as tc:
        _emit(tc, big, out)
    nc.compile()
    return nc


def get_nc():
    global _NC_CACHE
    if _NC_CACHE is None:
        _NC_CACHE = _build_nc()
    return _NC_CACHE


def _pack(arrs):
    """Pack the five f32 shards into one bf16 [NCORES, P, TOT] tensor with
    per-tile blocks [gt_r | gt_a | pred_r | pred_a | conf]."""
    flat = {nm: a.reshape(NCORES, P, COLS) for nm, a in arrs.items()}
    big = np.empty((NCORES, P, TOT), BF16NP)
    off = 0
    q0 = 0
    for w in WS:
        for nm in PACK_ORDER:
            big[:, :, off : off + w] = flat[nm][:, :, q0 : q0 + w].astype(BF16NP)
            off += w
        q0 += w
    return big


def _reference_loss_numpy(gt, pred, conf):
    """Exact numpy replica of the reference _get_loss (fallback path)."""
    n = gt.size
    gt = gt.reshape(-1).astype(np.float32)
    pred = pred.reshape(-1).astype(np.float32)
    conf = conf.reshape(-1).astype(np.float32)
    pos = (gt > POS_MIN).astype(np.float32)
    pos_num = np.float32(pos.sum(dtype=np.float32))
    neg_num = np.float32(min(np.float32(n) - pos_num, np.float32(NEG_RATIO) * pos_num))
    loss = (gt - pred) ** 2 * conf
    pos_loss_sum = np.float32((loss * pos).sum(dtype=np.float32))
    neg_loss = loss * (1.0 - pos)
    k = int(neg_num)
    sorted_neg = np.sort(neg_loss)[::-1]
    topk = np.float32(sorted_neg[:k].sum(dtype=np.float32))
    return float((topk + pos_loss_sum) / (neg_num + pos_num))


def kernel(**inputs):
    global LAST_RESULTS
    nc = get_nc()
    arrs = {
        nm: np.ascontiguousarray(np.asarray(inputs[nm], dtype=np.float32))
        for nm in NAMES
    }
    big = _pack(arrs)
    in_maps = [{"big": big[i]} for i in range(NCORES)]
    try:
        res = run_bass_kernel_spmd(nc, in_maps, core_ids=list(range(NCORES)))
    except Exception:
        # one retry: a previously-wedged device occasionally fails the first
        # execution with NRT_EXEC_UNIT_UNRECOVERABLE and recovers on rerun
        res = run_bass_kernel_spmd(nc, in_maps, core_ids=list(range(NCORES)))
    LAST_RESULTS = res
    accs = np.stack(
        [np.asarray(r["out"], dtype=np.float64).reshape(2, CHUNK) for r in res.results]
    )
    sums = accs.sum(axis=(0, 2))  # (2,) [region, affinity]
    n = float(N_FULL)
    total = 0.0
    specs = (
        (sums[0], "gt_region", "pred_region"),
        (sums[1], "gt_affinity", "pred_affinity"),
    )
    for l_sum, gt_nm, pr_nm in specs:
        # Branch decision only (O(n) boolean count, host): which arm the
        # reference's min() takes. The heavy loss reduction ran on device.
        pos_num = float(np.count_nonzero(arrs[gt_nm] > POS_MIN))
        neg_avail = n - pos_num
        if neg_avail <= NEG_RATIO * pos_num:
            # min() picks the full negative count -> top-k sums every negative
            total += l_sum / n
        else:
            total += _reference_loss_numpy(arrs[gt_nm], arrs[pr_nm], arrs["conf_map"])
    return np.float32(total)


# revision 46
# speedup vs baseline: 1.0719x; 1.0719x over previous
"""OHEM loss (region + affinity) on Trainium2 — 8 NeuronCores, SPMD data-parallel.

Math: for each pair (gt, pred) with shared conf_map,
    loss = (gt - pred)^2 * conf_map
    pos  = gt > 0.1 ; pos_num = sum(pos)
    neg_num = min(n - pos_num, 3 * pos_num)
    result  = (topk(neg_loss, neg_num).sum() + (loss*pos).sum()) / (neg_num + pos_num)
When neg_num == n - pos_num (the min picks the negative count, true whenever
pos fraction >= 0.25), the top-k covers every negative element, so
result == loss.sum() / n exactly. The device computes the per-shard
sum(loss) partials; the host combines them in float64, decides the min()
branch with a cheap boolean count, and falls back to an exact numpy
evaluation in the (never-taken-for-this-distribution) other branch.

Device-side design (memory-bound kernel, so bf16 halves HBM traffic):
- Host packs the five f32 shards into ONE bf16 HBM tensor per core,
  [128, sum(5*W)] with per-tile blocks [gt_r | gt_a | pred_r | pred_a | conf].
- One HWDGE DMA per tile (sync-engine trigger; all queued upfront), giving
  ~11.5KB per-partition packets — the per-packet sweet spot (>16KB packets
  drop per-engine DMA rate ~20%, and SWDGE gpsimd triggers suffer
  descriptor-ring contention on SDMA engine 15).
- Compute per tile: one fused 2W-wide DVE tensor_sub (bf16 2x mode) covers
  both pairs; pair0 squares on Act while pair1 squares on DVE (keeps the
  greedy scheduler fed with ready Vector work so it never serializes on the
  Act round-trip); DVE tensor_mul (2x) applies conf; the idle TensorE
  reduces along partitions via ones-matmul into an accumulating PSUM
  [1, CHUNK] per pair. All engines stay below the DMA stream time, so the
  kernel tracks the HBM roofline; the tapered tail tiles keep the post-DMA
  drain short.
- Output: the two PSUM accumulators, DMA'd out; host folds [2, CHUNK] x 8
  cores in f64.
"""

import os
import sys

import numpy as np

for _p in ("/opt/trn_rl_repo", os.path.expanduser("~/.axon_site/_ro/trn_rl_repo")):
    if os.path.isdir(_p) and _p not in sys.path:
        sys.path.insert(0, _p)

import ml_dtypes

import concourse.tile as tile
from concourse import bacc, mybir
from concourse.bass_utils import run_bass_kernel_spmd

B, CH, H, W = 16, 1, 768, 768
NCORES = 8
N_FULL = B * CH * H * W            # 9_437_184
N_CORE = N_FULL // NCORES          # 1_179_648
P = 128
COLS = N_CORE // P                 # 9216 free-dim columns per tensor per core
WS = (384, 1152, 1152, 1152, 1152, 1152, 1152, 1152, 768)
                                   # per-tile column widths (sum == COLS): small
                                   # tiles keep per-engine chains short; tapered
                                   # tail tiles minimize the post-DMA drain
CHUNK = 384                        # PSUM accumulator width (per-pair)
NT = len(WS)
TOT = 5 * COLS                     # packed columns per partition row (46080)
NEG_RATIO = 3.0
POS_MIN = 0.1
NAMES = ("gt_region", "pred_region", "gt_affinity", "pred_affinity", "conf_map")
# packed order: both gts adjacent, both preds adjacent -> one fused sub
PACK_ORDER = ("gt_region", "gt_affinity", "pred_region", "pred_affinity", "conf_map")
F32 = mybir.dt.float32
BF16 = mybir.dt.bfloat16
BF16NP = ml_dtypes.bfloat16

_NC_CACHE = None
LAST_RESULTS = None                # exposed for test harness profiling


def _emit(tc, big, out):
    nc = tc.nc
    wmax = max(WS)
    with (
        tc.tile_pool(name="io", bufs=9) as io_pool,
        tc.tile_pool(name="scr", bufs=4) as scr_pool,
        tc.tile_pool(name="cst", bufs=1) as cst_pool,
        tc.psum_pool(name="ps", bufs=1) as psum_pool,
    ):
        ones = cst_pool.tile([P, 1], BF16)
        nc.gpsimd.memset(ones[:], 1.0)
        ps_r = psum_pool.tile([1, CHUNK], F32, tag="ps0")
        ps_a = psum_pool.tile([1, CHUNK], F32, tag="ps1")
        ps = [ps_r, ps_a]
        started = [False, False]
        offs = [sum(5 * ww for ww in WS[:t]) for t in range(NT)]

        def load(t):
            w = WS[t]
            blk = io_pool.tile([P, 5 * wmax], BF16, tag="blk", name=f"blk{t}")
            # HWDGE (sync-engine trigger): avoids the SWDGE descriptor-ring
            # contention that slows SDMA engine 15 ~20%. Per-partition rows of
            # 5*w*2 = ~11.5KB are the per-packet DMA sweet spot.
            nc.sync.dma_start(blk[:, : 5 * w], big[:, offs[t] : offs[t] + 5 * w])
            return blk

        def sub(t, blk):
            w = WS[t]
            d = scr_pool.tile([P, 2 * wmax], BF16, tag="d", name=f"d{t}")
            nc.vector.tensor_sub(d[:, : 2 * w], blk[:, : 2 * w], blk[:, 2 * w : 4 * w])
            return d

        def squares(t, d):
            w = WS[t]
            out = []
            for pi in range(2):
                s = scr_pool.tile([P, wmax], BF16, tag=f"s{pi}", name=f"s{pi}_{t}")
                dp = d[:, pi * w : (pi + 1) * w]
                if pi == 0:
                    # pair0 squares on Act; pair1 squares on DVE. The DVE-side
                    # square+muls (~1520ns) cover Act's square latency
                    # (~1254ns), so the greedy scheduler always has ready
                    # Vector work and never serializes on the Act round-trip.
                    nc.scalar.square(s[:, :w], dp)
                else:
                    nc.vector.tensor_mul(s[:, :w], dp, dp)
                out.append(s)
            return out

        # 3-stage software pipeline (sub -> squares -> muls+matmuls), emitted
        # stage-shifted so each op's inputs were produced a full tile earlier:
        # the scheduler's per-engine queues then never stall on cross-engine
        # latency (which otherwise locks the cadence to the sub->square->mul
        # chain instead of the DMA rate).
        # all tiles buffered (io bufs == NT): queue every load upfront so the
        # HWDGE ring is never starved by trigger dependencies
        blks, ds, ss = {}, {}, {}
        for t in range(NT):
            blks[t] = load(t)
        ds[0] = sub(0, blks[0])
        ss[0] = squares(0, ds[0])
        for t, w in enumerate(WS):
            if t + 1 < NT:
                ds[t + 1] = sub(t + 1, blks[t + 1])
                ss[t + 1] = squares(t + 1, ds[t + 1])
            conf = blks[t][:, 4 * w : 5 * w]
            for pi in range(2):
                s = ss[t][pi]
                l = scr_pool.tile([P, wmax], BF16, tag=f"l{pi}", name=f"l{pi}_{t}")
                nc.vector.tensor_mul(l[:, :w], s[:, :w], conf)
                nchunk = w // CHUNK
                for j in range(nchunk):
                    last = t == NT - 1 and j == nchunk - 1
                    nc.tensor.matmul(
                        ps[pi][:],
                        ones[:],
                        l[:, j * CHUNK : (j + 1) * CHUNK],
                        start=not started[pi],
                        stop=last,
                        skip_group_check=True,
                    )
                    started[pi] = True
        fin = cst_pool.tile([1, 2 * CHUNK], F32, tag="fin")
        nc.vector.tensor_copy(fin[:, 0:CHUNK], ps[0][:])
        nc.scalar.copy(fin[:, CHUNK : 2 * CHUNK], ps[1][:])
        nc.sync.dma_start(out[:], fin[:])


def _build_nc():
    nc = bacc.Bacc("TRN2", target_bir_lowering=False, debug=False, num_devices=NCORES)
    big = nc.dram_tensor("big", [P, TOT], BF16, kind="ExternalInput").ap()
    out = nc.dram_tensor("out", [1, 2 * CHUNK], F32, kind="ExternalOutput").ap()
    with tile.TileContext(nc) as tc:
        _emit(tc, big, out)
    nc.compile()
    return nc


def get_nc():
    global _NC_CACHE
    if _NC_CACHE is None:
        _NC_CACHE = _build_nc()
    return _NC_CACHE


def _pack(arrs):
    """Pack the five f32 shards into one bf16 [NCORES, P, TOT] tensor with
    per-tile blocks [gt_r | gt_a | pred_r | pred_a | conf]."""
    flat = {nm: a.reshape(NCORES, P, COLS) for nm, a in arrs.items()}
    big = np.empty((NCORES, P, TOT), BF16NP)
    off = 0
    q0 = 0
    for w in WS:
        for nm in PACK_ORDER:
            big[:, :, off : off + w] = flat[nm][:, :, q0 : q0 + w].astype(BF16NP)
            off += w
        q0 += w
    return big


def _reference_loss_numpy(gt, pred, conf):
    """Exact numpy replica of the reference _get_loss (fallback path)."""
    n = gt.size
    gt = gt.reshape(-1).astype(np.float32)
    pred = pred.reshape(-1).astype(np.float32)
    conf = conf.reshape(-1).astype(np.float32)
    pos = (gt > POS_MIN).astype(np.float32)
    pos_num = np.float32(pos.sum(dtype=np.float32))
    neg_num = np.float32(min(np.float32(n) - pos_num, np.float32(NEG_RATIO) * pos_num))
    loss = (gt - pred) ** 2 * conf
    pos_loss_sum = np.float32((loss * pos).sum(dtype=np.float32))
    neg_loss = loss * (1.0 - pos)
    k = int(neg_num)
    sorted_neg = np.sort(neg_loss)[::-1]
    topk = np.float32(sorted_neg[:k].sum(dtype=np.float32))
    return float((topk + pos_loss_sum) / (neg_num + pos_num))


def kernel(**inputs):
    global LAST_RESULTS
    nc = get_nc()
    arrs = {
        nm: np.ascontiguousarray(np.asarray(inputs[nm], dtype=np.float32))
        for nm in NAMES
    }
    big = _pack(arrs)
    in_maps = [{"big": big[i]} for i in range(NCORES)]
    try:
        res = run_bass_kernel_spmd(nc, in_maps, core_ids=list(range(NCORES)))
    except Exception:
        # one retry: a previously-wedged device occasionally fails the first
        # execution with NRT_EXEC_UNIT_UNRECOVERABLE and recovers on rerun
        res = run_bass_kernel_spmd(nc, in_maps, core_ids=list(range(NCORES)))
    LAST_RESULTS = res
    accs = np.stack(
        [np.asarray(r["out"], dtype=np.float64).reshape(2, CHUNK) for r in res.results]
    )
    sums = accs.sum(axis=(0, 2))  # (2,) [region, affinity]
    n = float(N_FULL)
    total = 0.0
    specs = (
        (sums[0], "gt_region", "pred_region"),
        (sums[1], "gt_affinity", "pred_affinity"),
    )
    for l_sum, gt_nm, pr_nm in specs:
        # Branch decision only (O(n) boolean count, host): which arm the
        # reference's min() takes. The heavy loss reduction ran on device.
        pos_num = float(np.count_nonzero(arrs[gt_nm] > POS_MIN))
        neg_avail = n - pos_num
        if neg_avail <= NEG_RATIO * pos_num:
            # min() picks the full negative count -> top-k sums every negative
            total += l_sum / n
        else:
            total += _reference_loss_numpy(arrs[gt_nm], arrs[pr_nm], arrs["conf_map"])
    return np.float32(total)


# revision 47
# speedup vs baseline: 1.0904x; 1.0172x over previous
"""OHEM loss (region + affinity) on Trainium2 — 8 NeuronCores, SPMD data-parallel.

Math: for each pair (gt, pred) with shared conf_map,
    loss = (gt - pred)^2 * conf_map
    pos  = gt > 0.1 ; pos_num = sum(pos)
    neg_num = min(n - pos_num, 3 * pos_num)
    result  = (topk(neg_loss, neg_num).sum() + (loss*pos).sum()) / (neg_num + pos_num)
When neg_num == n - pos_num (the min picks the negative count, true whenever
pos fraction >= 0.25), the top-k covers every negative element, so
result == loss.sum() / n exactly. The device computes the per-shard
sum(loss) partials; the host combines them in float64, decides the min()
branch with a cheap boolean count, and falls back to an exact numpy
evaluation in the (never-taken-for-this-distribution) other branch.

Device-side design (memory-bound kernel, so bf16 halves HBM traffic):
- Host packs the five f32 shards into ONE bf16 HBM tensor per core,
  [128, sum(5*W)] with per-tile blocks [gt_r | gt_a | pred_r | pred_a | conf].
- One HWDGE DMA per tile (sync-engine trigger; all queued upfront), giving
  ~11.5KB per-partition packets — the per-packet sweet spot (>16KB packets
  drop per-engine DMA rate ~20%, and SWDGE gpsimd triggers suffer
  descriptor-ring contention on SDMA engine 15).
- Compute per tile: one fused 2W-wide DVE tensor_sub (bf16 2x mode) covers
  both pairs; pair0 squares on Act while pair1 squares on DVE (keeps the
  greedy scheduler fed with ready Vector work so it never serializes on the
  Act round-trip); DVE tensor_mul (2x) applies conf; the idle TensorE
  reduces along partitions via ones-matmul into an accumulating PSUM
  [1, CHUNK] per pair. All engines stay below the DMA stream time, so the
  kernel tracks the HBM roofline; the tapered tail tiles keep the post-DMA
  drain short.
- Output: the two PSUM accumulators, DMA'd out; host folds [2, CHUNK] x 8
  cores in f64.
"""

import os
import sys

import numpy as np

for _p in ("/opt/trn_rl_repo", os.path.expanduser("~/.axon_site/_ro/trn_rl_repo")):
    if os.path.isdir(_p) and _p not in sys.path:
        sys.path.insert(0, _p)

import ml_dtypes

import concourse.tile as tile
from concourse import bacc, mybir
from concourse.bass_utils import run_bass_kernel_spmd

B, CH, H, W = 16, 1, 768, 768
NCORES = 8
N_FULL = B * CH * H * W            # 9_437_184
N_CORE = N_FULL // NCORES          # 1_179_648
P = 128
COLS = N_CORE // P                 # 9216 free-dim columns per tensor per core
WS = (384, 1152, 1152, 1152, 1152, 1152, 1152, 1152, 768)
                                   # per-tile column widths (sum == COLS): small
                                   # tiles keep per-engine chains short; tapered
                                   # tail tiles minimize the post-DMA drain
CHUNK = 384                        # PSUM accumulator width (per-pair)
NT = len(WS)
TOT = 5 * COLS                     # packed columns per partition row (46080)
NEG_RATIO = 3.0
POS_MIN = 0.1
NAMES = ("gt_region", "pred_region", "gt_affinity", "pred_affinity", "conf_map")
# packed order: both gts adjacent, both preds adjacent -> one fused sub
PACK_ORDER = ("gt_region", "gt_affinity", "pred_region", "pred_affinity", "conf_map")
F32 = mybir.dt.float32
BF16 = mybir.dt.bfloat16
BF16NP = ml_dtypes.bfloat16

_NC_CACHE = None
LAST_RESULTS = None                # exposed for test harness profiling


def _emit(tc, big, out):
    nc = tc.nc
    wmax = max(WS)
    with (
        tc.tile_pool(name="io", bufs=9) as io_pool,
        tc.tile_pool(name="scr", bufs=6) as scr_pool,
        tc.tile_pool(name="cst", bufs=1) as cst_pool,
        tc.psum_pool(name="ps", bufs=1) as psum_pool,
    ):
        ones = cst_pool.tile([P, 1], BF16)
        nc.gpsimd.memset(ones[:], 1.0)
        ps_r = psum_pool.tile([1, CHUNK], F32, tag="ps0")
        ps_a = psum_pool.tile([1, CHUNK], F32, tag="ps1")
        ps = [ps_r, ps_a]
        started = [False, False]
        offs = [sum(5 * ww for ww in WS[:t]) for t in range(NT)]

        def load(t):
            w = WS[t]
            blk = io_pool.tile([P, 5 * wmax], BF16, tag="blk", name=f"blk{t}")
            # HWDGE (sync-engine trigger): avoids the SWDGE descriptor-ring
            # contention that slows SDMA engine 15 ~20%. Per-partition rows of
            # 5*w*2 = ~11.5KB are the per-packet DMA sweet spot.
            nc.sync.dma_start(blk[:, : 5 * w], big[:, offs[t] : offs[t] + 5 * w])
            return blk

        def sub(t, blk):
            w = WS[t]
            d = scr_pool.tile([P, 2 * wmax], BF16, tag="d", name=f"d{t}")
            nc.vector.tensor_sub(d[:, : 2 * w], blk[:, : 2 * w], blk[:, 2 * w : 4 * w])
            return d

        def squares(t, d):
            w = WS[t]
            out = []
            for pi in range(2):
                s = scr_pool.tile([P, wmax], BF16, tag=f"s{pi}", name=f"s{pi}_{t}")
                dp = d[:, pi * w : (pi + 1) * w]
                if pi == 0:
                    # pair0 squares on Act; pair1 squares on DVE. The DVE-side
                    # square+muls (~1520ns) cover Act's square latency
                    # (~1254ns), so the greedy scheduler always has ready
                    # Vector work and never serializes on the Act round-trip.
                    nc.scalar.square(s[:, :w], dp)
                else:
                    nc.vector.tensor_mul(s[:, :w], dp, dp)
                out.append(s)
            return out

        # 3-stage software pipeline (sub -> squares -> muls+matmuls), emitted
        # stage-shifted so each op's inputs were produced a full tile earlier:
        # the scheduler's per-engine queues then never stall on cross-engine
        # latency (which otherwise locks the cadence to the sub->square->mul
        # chain instead of the DMA rate).
        # all tiles buffered (io bufs == NT): queue every load upfront so the
        # HWDGE ring is never starved by trigger dependencies
        blks, ds, ss = {}, {}, {}
        for t in range(NT):
            blks[t] = load(t)
        ds[0] = sub(0, blks[0])
        ss[0] = squares(0, ds[0])
        for t, w in enumerate(WS):
            if t + 1 < NT:
                ds[t + 1] = sub(t + 1, blks[t + 1])
                ss[t + 1] = squares(t + 1, ds[t + 1])
            conf = blks[t][:, 4 * w : 5 * w]
            for pi in range(2):
                s = ss[t][pi]
                l = scr_pool.tile([P, wmax], BF16, tag=f"l{pi}", name=f"l{pi}_{t}")
                nc.vector.tensor_mul(l[:, :w], s[:, :w], conf)
                nchunk = w // CHUNK
                for j in range(nchunk):
                    last = t == NT - 1 and j == nchunk - 1
                    nc.tensor.matmul(
                        ps[pi][:],
                        ones[:],
                        l[:, j * CHUNK : (j + 1) * CHUNK],
                        start=not started[pi],
                        stop=last,
                        skip_group_check=True,
                    )
                    started[pi] = True
        fin = cst_pool.tile([1, 2 * CHUNK], F32, tag="fin")
        nc.vector.tensor_copy(fin[:, 0:CHUNK], ps[0][:])
        nc.scalar.copy(fin[:, CHUNK : 2 * CHUNK], ps[1][:])
        nc.sync.dma_start(out[:], fin[:])


def _build_nc():
    nc = bacc.Bacc("TRN2", target_bir_lowering=False, debug=False, num_devices=NCORES)
    big = nc.dram_tensor("big", [P, TOT], BF16, kind="ExternalInput").ap()
    out = nc.dram_tensor("out", [1, 2 * CHUNK], F32, kind="ExternalOutput").ap()
    with tile.TileContext(nc) as tc:
        _emit(tc, big, out)
    nc.compile()
    return nc


def get_nc():
    global _NC_CACHE
    if _NC_CACHE is None:
        _NC_CACHE = _build_nc()
    return _NC_CACHE


def _pack(arrs):
    """Pack the five f32 shards into one bf16 [NCORES, P, TOT] tensor with
    per-tile blocks [gt_r | gt_a | pred_r | pred_a | conf]."""
    flat = {nm: a.reshape(NCORES, P, COLS) for nm, a in arrs.items()}
    big = np.empty((NCORES, P, TOT), BF16NP)
    off = 0
    q0 = 0
    for w in WS:
        for nm in PACK_ORDER:
            big[:, :, off : off + w] = flat[nm][:, :, q0 : q0 + w].astype(BF16NP)
            off += w
        q0 += w
    return big


def _reference_loss_numpy(gt, pred, conf):
    """Exact numpy replica of the reference _get_loss (fallback path)."""
    n = gt.size
    gt = gt.reshape(-1).astype(np.float32)
    pred = pred.reshape(-1).astype(np.float32)
    conf = conf.reshape(-1).astype(np.float32)
    pos = (gt > POS_MIN).astype(np.float32)
    pos_num = np.float32(pos.sum(dtype=np.float32))
    neg_num = np.float32(min(np.float32(n) - pos_num, np.float32(NEG_RATIO) * pos_num))
    loss = (gt - pred) ** 2 * conf
    pos_loss_sum = np.float32((loss * pos).sum(dtype=np.float32))
    neg_loss = loss * (1.0 - pos)
    k = int(neg_num)
    sorted_neg = np.sort(neg_loss)[::-1]
    topk = np.float32(sorted_neg[:k].sum(dtype=np.float32))
    return float((topk + pos_loss_sum) / (neg_num + pos_num))


def kernel(**inputs):
    global LAST_RESULTS
    nc = get_nc()
    arrs = {
        nm: np.ascontiguousarray(np.asarray(inputs[nm], dtype=np.float32))
        for nm in NAMES
    }
    big = _pack(arrs)
    in_maps = [{"big": big[i]} for i in range(NCORES)]
    try:
        res = run_bass_kernel_spmd(nc, in_maps, core_ids=list(range(NCORES)))
    except Exception:
        # one retry: a previously-wedged device occasionally fails the first
        # execution with NRT_EXEC_UNIT_UNRECOVERABLE and recovers on rerun
        res = run_bass_kernel_spmd(nc, in_maps, core_ids=list(range(NCORES)))
    LAST_RESULTS = res
    accs = np.stack(
        [np.asarray(r["out"], dtype=np.float64).reshape(2, CHUNK) for r in res.results]
    )
    sums = accs.sum(axis=(0, 2))  # (2,) [region, affinity]
    n = float(N_FULL)
    total = 0.0
    specs = (
        (sums[0], "gt_region", "pred_region"),
        (sums[1], "gt_affinity", "pred_affinity"),
    )
    for l_sum, gt_nm, pr_nm in specs:
        # Branch decision only (O(n) boolean count, host): which arm the
        # reference's min() takes. The heavy loss reduction ran on device.
        pos_num = float(np.count_nonzero(arrs[gt_nm] > POS_MIN))
        neg_avail = n - pos_num
        if neg_avail <= NEG_RATIO * pos_num:
            # min() picks the full negative count -> top-k sums every negative
            total += l_sum / n
        else:
            total += _reference_loss_numpy(arrs[gt_nm], arrs[pr_nm], arrs["conf_map"])
    return np.float32(total)
